# revision 13
# baseline (speedup 1.0000x reference)
"""Causal self-attention (B=4, T=2048, C=1024, H=16) on 8 TRN2 NeuronCores.

Sharding: tensor-parallel over heads. Core c owns heads (2c, 2c+1) for all
batches: QKV projections are column-sharded, attention is embarrassingly
parallel over (batch, head), out_proj is contraction-sharded and the host
sums the 8 partial outputs (the unshard step for a contraction shard).

Single-core schedule: a software pipeline over batches.  For each batch b:
projections+RoPE for b's 2048 tokens, then attention (both heads), with the
NEXT batch's projection work and the PREVIOUS batch's out_proj tiles
interleaved into the attention kb-loop as "filler" so the PE never idles
while the ACT engine runs the softmax exp stream.

Key device-level tricks:
  - S^T tiles for the two heads are computed by a row-tiled PAIR of matmuls
    (tile_position (0,0) and (64,0)): each head contracts over its own 64
    partitions, so both matmuls run concurrently on the PE array, writing
    the two halves of one [128, 1024] PSUM tile.
  - exp runs on ACT over the joint [128, 1024] tile (one instruction for
    both heads) with scale=1/8; max-subtraction is skipped (|S|/8 <= ~3.2
    for this operator).  The causal staircase is a 0/1 bf16 mask multiply
    on DVE, column-trimmed to the triangle's actual width.
  - V is transposed token-major 128x128 (both heads at once) on the PE; the
    vtm layout [128, gkb, 2, 66] keeps each head's 64 dims + a ones column
    contiguous, so the PV matmul (lhsT = vtm[:, gkb, j, 0:65]) emits both
    O^T rows and the softmax denominator in one accumulation.
  - out_proj partials are written bf16 (halves the output DMA); the host
    sums the 8 partials in fp32, adds bo, and transposes back.
  - ACT does exp only (plus half the out_proj PSUM drains); projections'
    RoPE outputs, V bias, O drains and normalization run on DVE/Pool.
"""

import numpy as np
import ml_dtypes

import concourse.bass as bass
import concourse.mybir as mybir
import concourse.tile as tile
from concourse import bacc
from concourse.bass_utils import run_bass_kernel_spmd
from concourse.masks import make_identity

BF16 = mybir.dt.bfloat16
F32 = mybir.dt.float32
AT = mybir.ActivationFunctionType
OP = mybir.AluOpType

B, T, C, H = 4, 2048, 1024, 16
DH = 64
BT = B * T            # 8192
NCORES = 8
NKB = T // 128        # 16 k-blocks per batch
NQT = T // 512        # 4 q-tiles per batch

_NC = None            # cached compiled Bass module


def _build_nc(repeat=1, phases="BCD"):
    nc = bacc.Bacc("TRN2", target_bir_lowering=False, debug=False)

    xT = nc.declare_dram_parameter("xT", [C, BT], BF16, isOutput=False)
    wq = nc.declare_dram_parameter("wq", [C, 128], BF16, isOutput=False)
    wk = nc.declare_dram_parameter("wk", [C, 128], BF16, isOutput=False)
    wv = nc.declare_dram_parameter("wv", [C, 128], BF16, isOutput=False)
    wo = nc.declare_dram_parameter("wo", [128, C], BF16, isOutput=False)
    bq = nc.declare_dram_parameter("bq", [128, 1], F32, isOutput=False)
    bk = nc.declare_dram_parameter("bk", [128, 1], F32, isOutput=False)
    bv = nc.declare_dram_parameter("bv", [128, 1], F32, isOutput=False)
    csa = nc.declare_dram_parameter("csa", [128, T], F32, isOutput=False)
    csb = nc.declare_dram_parameter("csb", [128, T], F32, isOutput=False)
    msk = nc.declare_dram_parameter("msk", [128, 4, 512], BF16, isOutput=False)
    ca = nc.declare_dram_parameter("ca", [128, 128], BF16, isOutput=False)
    cb = nc.declare_dram_parameter("cb", [128, 128], BF16, isOutput=False)
    outT = nc.declare_dram_parameter("outT", [C, BT], BF16, isOutput=True)

    from contextlib import ExitStack
    from collections import deque
    with tile.TileContext(nc) as tc, ExitStack() as ctx:
        const = ctx.enter_context(tc.tile_pool(name="const", bufs=1))
        xpool = ctx.enter_context(tc.tile_pool(name="xpool", bufs=3))
        ptp = ctx.enter_context(tc.tile_pool(name="ptp", bufs=4))
        rtmp = ctx.enter_context(tc.tile_pool(name="rtmp", bufs=4))
        opool = ctx.enter_context(tc.tile_pool(name="opool", bufs=2))
        small = ctx.enter_context(tc.tile_pool(name="small", bufs=3))
        otp = ctx.enter_context(tc.tile_pool(name="otp", bufs=4))
        psA = ctx.enter_context(tc.tile_pool(name="psA", bufs=2, space="PSUM"))
        psS = ctx.enter_context(tc.tile_pool(name="psS", bufs=2, space="PSUM"))
        psO = ctx.enter_context(tc.tile_pool(name="psO", bufs=2, space="PSUM"))

        # ---- constants ----
        wq_sb = const.tile([128, 8, 128], BF16, tag="wq")
        nc.sync.dma_start(out=wq_sb, in_=wq.rearrange("(kb p) m -> p kb m", p=128))
        wk_sb = const.tile([128, 8, 128], BF16, tag="wk")
        nc.sync.dma_start(out=wk_sb, in_=wk.rearrange("(kb p) m -> p kb m", p=128))
        wv_sb = const.tile([128, 8, 128], BF16, tag="wv")
        nc.sync.dma_start(out=wv_sb, in_=wv.rearrange("(kb p) m -> p kb m", p=128))
        wo_sb = const.tile([128, 8, 128], BF16, tag="wo")
        nc.sync.dma_start(out=wo_sb, in_=wo.rearrange("p (ob m) -> p ob m", m=128))
        csa_sb = const.tile([128, T], F32, tag="csa")
        nc.sync.dma_start(out=csa_sb, in_=csa[:, :])
        csb_sb = const.tile([128, T], F32, tag="csb")
        nc.sync.dma_start(out=csb_sb, in_=csb[:, :])
        msk_sb = const.tile([128, 4, 512], BF16, tag="msk")
        nc.sync.dma_start(out=msk_sb, in_=msk[:, :, :])
        ca_sb = const.tile([128, 128], BF16, tag="ca")
        nc.sync.dma_start(out=ca_sb, in_=ca[:, :])
        cb_sb = const.tile([128, 128], BF16, tag="cb")
        nc.sync.dma_start(out=cb_sb, in_=cb[:, :])
        bq_sb = const.tile([128, 1], F32, tag="bq")
        nc.sync.dma_start(out=bq_sb, in_=bq[:, :])
        bk_sb = const.tile([128, 1], F32, tag="bk")
        nc.sync.dma_start(out=bk_sb, in_=bk[:, :])
        bv_sb = const.tile([128, 1], F32, tag="bv")
        nc.sync.dma_start(out=bv_sb, in_=bv[:, :])

        ident = const.tile([128, 128], BF16, tag="id")
        make_identity(nc, ident)

        QT = const.tile([128, BT], BF16, tag="QT")
        KT = const.tile([128, BT], BF16, tag="KT")
        VT = const.tile([128, BT], BF16, tag="VT")
        yT = const.tile([128, BT], BF16, tag="yT")
        # token-major V: [k-part, gkb, head, 64 dims + ones + pad]
        vtm = const.tile([128, B * NKB, 2, 66], BF16, tag="vtm")
        nc.vector.memset(vtm[:, :, :, 64:65], 1.0)

        def emit_body():
            # ---------- phase B: projections + RoPE + V transpose ----------
            def b_items(b):
                """Generator of filler closures for batch b's projections.

                All tile() allocations happen inside the closures (at pop
                time) so pool-slot WAR deps follow emission order; tiles
                shared between closures of one token-tile pass via `cell`.
                """
                for tt in range(NQT * b, NQT * b + NQT):
                    ts_ = slice(tt * 512, tt * 512 + 512)
                    pos = slice((tt % 4) * 512, (tt % 4) * 512 + 512)
                    cell = {}

                    def dma_x(cell=cell, ts_=ts_, tt=tt):
                        xt = xpool.tile([128, 8, 512], BF16, tag="xt",
                                        name=f"xt_{tt}")
                        cell["xt"] = xt
                        # Alternate the two HWDGE trigger queues so x loads
                        # overlap each other and never all serialize behind
                        # the outT stores or the big constant DMAs.
                        eng = nc.scalar if tt % 2 == 0 else nc.sync
                        eng.dma_start(
                            out=xt,
                            in_=xT.rearrange("(kb p) m -> p kb m", p=128)[:, :, ts_])
                    yield dma_x

                    for w_sb, b_sb, dstT, rope in (
                        (wq_sb, bq_sb, QT, True),
                        (wk_sb, bk_sb, KT, True),
                        (wv_sb, bv_sb, VT, False),
                    ):
                        pk = f"pp_{dstT.name}"

                        def mm4(lo, cell=cell, w_sb=w_sb, pk=pk, tt=tt):
                            if lo == 0:
                                cell[pk] = psA.tile([128, 512], F32, tag="proj",
                                                    name=f"{pk}_{tt}")
                            pp = cell[pk]
                            xt = cell["xt"]
                            for kb in range(lo, lo + 4):
                                nc.tensor.matmul(pp, w_sb[:, kb, :], xt[:, kb, :],
                                                 start=(kb == 0), stop=(kb == 7))
                        yield (lambda mm4=mm4: mm4(0))
                        yield (lambda mm4=mm4: mm4(4))

                        if not rope:
                            def v_epi(cell=cell, b_sb=b_sb, ts_=ts_, pk=pk):
                                nc.vector.tensor_scalar_add(VT[:, ts_], cell[pk],
                                                            b_sb[:, 0:1])
                            yield v_epi
                        else:
                            def r_epi(cell=cell, b_sb=b_sb, dstT=dstT, ts_=ts_,
                                      pos=pos, tt=tt, pk=pk):
                                pp = cell[pk]
                                ta = rtmp.tile([128, 512], BF16, tag="ta",
                                               name=f"ta_{tt}_{dstT.name}")
                                tb = rtmp.tile([128, 512], BF16, tag="tb",
                                               name=f"tb_{tt}_{dstT.name}")
                                nc.vector.scalar_tensor_tensor(
                                    out=ta, in0=pp, scalar=b_sb[:, 0:1],
                                    in1=csa_sb[:, pos], op0=OP.add, op1=OP.mult)
                                nc.vector.scalar_tensor_tensor(
                                    out=tb, in0=pp, scalar=b_sb[:, 0:1],
                                    in1=csb_sb[:, pos], op0=OP.add, op1=OP.mult)
                                rp = psA.tile([128, 512], F32, tag="proj",
                                              name=f"rp_{tt}_{dstT.name}")
                                nc.tensor.matmul(rp, ca_sb, ta, start=True,
                                                 stop=False)
                                nc.tensor.matmul(rp, cb_sb, tb, start=False,
                                                 stop=True)
                                nc.vector.tensor_copy(dstT[:, ts_], rp)
                            yield r_epi

                    # V^T -> token-major 128x128 transposes (both heads)
                    for half in range(2):
                        def v_tr(tt=tt, half=half):
                            tr = psA.tile([128, 512], F32, tag="proj",
                                          name=f"tr_{tt}_{half}")
                            trb = tr.bitcast(BF16)  # [128, 1024] bf16 view
                            for s in range(2):
                                sub = half * 2 + s
                                gkb = tt * 4 + sub
                                col = slice(tt * 512 + sub * 128,
                                            tt * 512 + sub * 128 + 128)
                                nc.tensor.transpose(
                                    trb[:, 128 * s:128 * s + 128],
                                    VT[:, col], ident)
                                nc.vector.tensor_copy(
                                    vtm[:, gkb, :, 0:64],
                                    trb[:, 128 * s:128 * s + 128]
                                    .rearrange("p (j d) -> p j d", j=2))
                        yield v_tr

            # ---------- phase D: out_proj partials (per finished q-tile) ----
            def d_tile(tt):
                for ob in range(8):
                    if True:
                        def op_item(ob=ob, tt=tt):
                            ts_ = slice(tt * 512, tt * 512 + 512)
                            pp = psA.tile([128, 512], F32, tag="proj",
                                          name=f"op_{tt}_{ob}")
                            nc.tensor.matmul(pp, wo_sb[:, ob, :], yT[:, ts_],
                                             start=True, stop=True)
                            ot = otp.tile([128, 512], BF16, tag="ot",
                                          name=f"ot_{tt}_{ob}")
                            if (ob + tt) % 2 == 0:
                                nc.vector.tensor_copy(ot, pp)
                            else:
                                nc.scalar.copy(ot, pp)
                            nc.sync.dma_start(
                                out=outT[ob * 128:(ob + 1) * 128, ts_], in_=ot)
                        yield op_item

            # ---------- phase C: attention for batch b ----------
            def attention(b, fillB, fillD):
                def pop2():
                    # one out_proj item and one projection item per slot
                    if fillD:
                        fillD.popleft()()
                    if fillB:
                        fillB.popleft()()
                    elif fillD:
                        fillD.popleft()()

                for qt in range(NQT):
                    nkb = 4 * qt + 4
                    qsl = slice(b * T + qt * 512, b * T + qt * 512 + 512)
                    ops = [psO.tile([128, 512], F32, tag="o",
                                    name=f"o_{b}_{qt}_{j}") for j in range(2)]
                    sps = {}

                    def qoff(kb):
                        # on diagonal tiles, query columns below 128*jj are
                        # fully masked: S/exp/PV skip them entirely
                        jj = kb - (nkb - 4)
                        return 128 * jj if jj >= 1 else 0

                    def emit_S(kb):
                        ksl = slice(b * T + kb * 128, b * T + kb * 128 + 128)
                        qo = qoff(kb)
                        qs = slice(qsl.start + qo, qsl.stop)
                        sp = psS.tile([128, 1024], F32, tag="s",
                                      name=f"s_{b}_{qt}_{kb}")
                        nc.tensor.matmul(sp[:, qo:512], KT[0:64, ksl],
                                         QT[0:64, qs], start=True, stop=True,
                                         tile_position=(0, 0))
                        nc.tensor.matmul(sp[:, 512 + qo:1024], KT[64:128, ksl],
                                         QT[64:128, qs], start=True, stop=True,
                                         tile_position=(64, 0))
                        sps[kb] = sp

                    emit_S(0)
                    for kb in range(nkb):
                        # next S-pair ahead of this kb's PV: the PE works on
                        # S(kb+1) while ACT runs exp(kb)
                        if kb + 1 < nkb:
                            emit_S(kb + 1)
                        sp = sps.pop(kb)
                        qo = qoff(kb)
                        pt = ptp.tile([128, 1024], BF16, tag="pt",
                                      name=f"pt_{b}_{qt}_{kb}")
                        if qo >= 256:
                            for j in range(2):
                                o = 512 * j
                                nc.scalar.activation(pt[:, o + qo:o + 512],
                                                     sp[:, o + qo:o + 512],
                                                     AT.Exp, scale=0.125)
                        else:
                            nc.scalar.activation(pt, sp, AT.Exp, scale=0.125)
                        if kb >= nkb - 4:
                            jj = kb - (nkb - 4)
                            lim = slice(128 * jj, 128 * jj + 128)
                            for j in range(2):
                                o = 512 * j
                                nc.vector.tensor_tensor(
                                    out=pt[:, o + lim.start:o + lim.stop],
                                    in0=pt[:, o + lim.start:o + lim.stop],
                                    in1=msk_sb[:, jj, lim], op=OP.mult)
                        gkb = b * NKB + kb
                        for j in range(2):
                            nc.tensor.matmul(
                                ops[j][0:65, qo:512], vtm[:, gkb, j, 0:65],
                                pt[:, 512 * j + qo:512 * j + 512],
                                start=(kb == 0), stop=(kb == nkb - 1))
                        pop2()
                    # normalize: O / denom -> yT (bf16)
                    for j in range(2):
                        hsl = slice(64 * j, 64 * j + 64)
                        osb = opool.tile([65, 512], F32, tag="osb",
                                         name=f"osb_{b}_{qt}_{j}")
                        nc.vector.tensor_copy(osb, ops[j][0:65, :])
                        recip = small.tile([1, 512], F32, tag="rc",
                                           name=f"rc_{b}_{qt}_{j}")
                        nc.vector.reciprocal(recip, osb[64:65, :])
                        rbt = small.tile([64, 512], F32, tag="rb",
                                         name=f"rb_{b}_{qt}_{j}")
                        nc.gpsimd.partition_broadcast(rbt, recip)
                        nc.vector.tensor_tensor(
                            out=yT[hsl, qsl], in0=osb[0:64, :], in1=rbt,
                            op=OP.mult)
                    # this q-tile's out_proj partials are now computable
                    fillD.extend(d_tile(NQT * b + qt))
                    pop2()
                    pop2()

            fillB = deque()
            fillD = deque()
            for it in b_items(0):
                it()
            for b in range(B):
                if b + 1 < B:
                    fillB.extend(b_items(b + 1))
                attention(b, fillB, fillD)
                # next batch's projections must fully precede its attention
                while fillB:
                    fillB.popleft()()
            while fillD:
                fillD.popleft()()

        for _ in range(repeat):
            emit_body()

    nc.compile()
    return nc


def _get_nc():
    global _NC
    if _NC is None:
        _NC = _build_nc()
    return _NC


def _prep_in_maps(x, Wq, bq, Wk, bk, Wv, bv, Wo, bo):
    bf = ml_dtypes.bfloat16
    # x^T, bf16-rounded (matches reference's x.astype(bf16) exactly)
    xT = np.ascontiguousarray(
        np.asarray(x, np.float32).reshape(BT, C).astype(bf).T
    )

    # RoPE caches; rows [cos|sin|cos|sin] and [sin|cos|sin|cos]
    inv = (1.0 / 10000.0 ** (np.arange(0, DH, 2, dtype=np.float64) / DH))
    pos = np.arange(T, dtype=np.float64)
    fr = np.outer(pos, inv)                      # [T, 32]
    cosT = np.cos(fr).T.astype(np.float32)       # [32, T]
    sinT = np.sin(fr).T.astype(np.float32)
    csa = np.ascontiguousarray(np.concatenate([cosT, sinT, cosT, sinT], 0))
    csb = np.ascontiguousarray(np.concatenate([sinT, cosT, sinT, cosT], 0))

    # causal staircase masks for the 4 diagonal k-blocks of each q-tile
    ki = np.arange(128)[:, None]
    qi = np.arange(512)[None, :]
    msk = np.stack(
        [(qi >= 128 * jj + ki) for jj in range(4)], axis=1
    ).astype(bf)                                  # [128, 4, 512]

    # RoPE combine matrices: rot = Ca^T t_a + Cb^T t_b
    ca = np.zeros((128, 128), np.float32)
    cb = np.zeros((128, 128), np.float32)
    for base in (0, 64):
        for m in range(32):
            ca[base + m, base + m] = 1.0          # E*cos
            ca[base + m + 32, base + m] = -1.0    # -O*sin
            cb[base + m, base + m + 32] = 1.0     # E*sin
            cb[base + m + 32, base + m + 32] = 1.0  # O*cos
    ca = ca.astype(bf)
    cb = cb.astype(bf)

    perm = np.concatenate([np.arange(0, DH, 2), np.arange(1, DH, 2)])
    Wq = np.asarray(Wq, np.float32)
    Wk = np.asarray(Wk, np.float32)
    Wv = np.asarray(Wv, np.float32)
    Wo = np.asarray(Wo, np.float32)
    bq = np.asarray(bq, np.float32)
    bk = np.asarray(bk, np.float32)
    bv = np.asarray(bv, np.float32)

    in_maps = []
    for c in range(NCORES):
        h0, h1 = 2 * c, 2 * c + 1
        cols = np.concatenate([DH * h0 + perm, DH * h1 + perm])
        in_maps.append({
            "xT": xT,
            "wq": np.ascontiguousarray(Wq[:, cols].astype(bf)),
            "wk": np.ascontiguousarray(Wk[:, cols].astype(bf)),
            "wv": np.ascontiguousarray(Wv[:, 128 * c:128 * c + 128].astype(bf)),
            "wo": np.ascontiguousarray(Wo[128 * c:128 * c + 128, :].astype(bf)),
            "bq": np.ascontiguousarray(bq[cols].reshape(128, 1)),
            "bk": np.ascontiguousarray(bk[cols].reshape(128, 1)),
            "bv": np.ascontiguousarray(
                bv[128 * c:128 * c + 128].reshape(128, 1)),
            "csa": csa, "csb": csb, "msk": msk, "ca": ca, "cb": cb,
        })
    return in_maps


def _gather(results, bo):
    acc = results[0]["outT"].astype(np.float32)
    for c in range(1, NCORES):
        acc = acc + results[c]["outT"].astype(np.float32)
    out = acc.T.reshape(B, T, C) + np.asarray(bo, np.float32)
    return np.ascontiguousarray(out.astype(np.float32))


def kernel(x, Wq, bq, Wk, bk, Wv, bv, Wo, bo):
    nc = _get_nc()
    in_maps = _prep_in_maps(x, Wq, bq, Wk, bk, Wv, bv, Wo, bo)
    res = run_bass_kernel_spmd(nc, in_maps, list(range(NCORES)))
    return _gather(res.results, bo)
